# revision 16
# baseline (speedup 1.0000x reference)
"""Trainium2 Bass kernel for linear attention (elu+1 feature map).

Reference computation (B=4, N=M=8192, C=512, H=8, D=64):
    kv   = ref @ kv_w.T              -> k, v  [B,H,N,D]
    q    = tgt @ q_w.T               -> [B,H,M,D];  q,k -> elu(x)+1
    ctx  = sum_n k v^T per head      -> [B,H,D,D];  ksum = sum_n k
    x    = (q @ ctx) * SCALE / (1e-6 + q . ksum)
    out  = x @ proj_w.T + proj_b     -> [B,M,C]

Sharding: 8 cores = 4 batches x 2 row-halves; per-head state is
pair-AllReduced in two half-N slices (second slice on the tail).

Key restructurings vs the v-projection baseline:
  - v is never projected: ctx_h = (k_h^T ref) @ Wv_h^T.  Phase 1
    accumulates Gt[c, kd] = sum_n ref[n,c] k[n,kd] (fp8 DoubleRow,
    rhs = on-device fp8 k) and a ksum row via a ones-column stationary;
    the tiny per-head ctx^T is built from Gt right before the
    collective.  This kills the v matmuls, the v psum->sbuf copies,
    and the 129-wide ctx accumulation matmuls of the baseline.
  - The x matmul and out-projection are fused: with q' = q_elu * rec,
    out = q' @ U where U[(h,d),f] = sum_e ctx_h[d,e] proj_w[f,64h+e].
    U ([512,512] fp8) is built once post-collective; the per-chunk
    x matmuls and their psum drains disappear.
  - elu+1 = min(exp, WS) + max(x, 0): exp on ACT, max on DVE; the
    sbuf-only combine runs on GpSimd for k (writes fp8 directly) and
    on DVE at 2x-perf-mode [128,1024] for q (writes bf16).
  - The reciprocal's output feeds the E-broadcast matmul as float32r
    (1 cycle/row for free-dim 512), dropping the bf16-convert DVE op.
  - Scales: WS=48 feature scale, Gt8 = Gt*2^-9, U8 = U_ps*2^-2,
    E entries = SCALE*2^31, out drain = 2^-20/WS; host adds proj_b.
    All fp8 host packs clipped to +-240 (TRN e4m3 max; no saturation).
"""

import math
import re

import numpy as np
import ml_dtypes

import concourse.bass as bass
import concourse.mybir as mybir
from concourse import bacc
from concourse.tile import TileContext
from concourse.bass import ts
from concourse.bass_utils import run_bass_kernel_spmd
import concourse.dve_ops as dops
from concourse.dve_ops import DveOp
from concourse.dve_spec import Spec, Src0, Src1, C0, C1, select


def _make_dve_op(name, spec, subdim=False):
    """Register a custom DVE op, pinning its uop shas by probing lower()."""
    for o in dops.OPS:
        if o.name == name:
            return o
    if name not in dops._SUB_OPCODE_FOR_NAME:
        dops._SUB_OPCODE_FOR_NAME[name] = (
            max(dops._SUB_OPCODE_FOR_NAME.values()) + 1)
    shas = {}
    for ver in ("v3", "v4"):
        try:
            DveOp(name, spec, subdim=subdim, uops_sha={}).compile(ver)
        except ValueError as ex:
            shas[ver] = re.search(rf"{ver}: ([0-9a-f]+)", str(ex)).group(1)
    op = DveOp(name, spec, subdim=subdim, uops_sha=shas)
    dops.OPS.append(op)
    dops.CUSTOM_DVE_SPECS[name] = spec
    return op


# out = in0 < 0 ? in1 : in0 + s1  -- the whole scaled elu1 tail in one
# DVE op (in0 = pre-activation psum, in1 = WS*exp(in0/WS) from ACT).
ELU_SEL = _make_dve_op("ELU_SEL_ANT", Spec(
    body=select(Src0 < C0, Src1, Src0 + C1),
    reference=lambda in0, in1, s0, s1, imm2: np.where(
        in0 < s0, in1, in0.astype(np.float32) + s1).astype(np.float32),
))

B, N, M, C, H = 4, 8192, 8192, 512, 8
D = C // H
SCALE = D**-0.5
NCORES = 8
BF = mybir.dt.bfloat16
F32 = mybir.dt.float32
F32R = mybir.dt.float32r
F8 = mybir.dt.float8e4

WS = 48.0               # feature-map scale (k/q weights host-scaled by WS)
LN_WS = math.log(WS)
SG = 2.0**-9            # Gt psum -> fp8 drain scale
SU = 2.0**-2            # U psum -> fp8 drain scale
E0 = 2.0**31            # E-broadcast matrix value = SCALE * E0
DRAIN_S = 2.0**-20 / WS  # out-proj psum -> bf16 drain scale
REFW = 528              # ref8n free width: 512 ref cols + ones col + pad

_CACHE = {}


def build(R_ref, R_q, num_devices, replica_groups, lookahead=4):
    """Emit the SPMD graph. R_ref/R_q = rows of the ref/target shard."""
    P = 128
    KC = C // P          # 4 kd-tiles
    KCP = KC // 2        # 2 kd-pair-tiles (DoubleRow)
    NT1 = R_ref // P     # phase-1 row tiles
    NPR = NT1 // 2       # phase-1 row-tile pairs
    CH = 512             # phase-2 chunk rows
    NCH = R_q // CH
    RT = CH // P
    NPAIR = H // 2
    STATE = P * C + C    # collective payload: ctxT [128,512] | ksum tail
    DR = mybir.MatmulPerfMode.DoubleRow

    nc = bacc.Bacc("TRN2", target_bir_lowering=False, debug=False,
                   num_devices=num_devices)

    refT8 = nc.dram_tensor("refT8", [KCP, P, 2, R_ref], F8,
                           kind="ExternalInput")
    ref8n = nc.dram_tensor("ref8n", [NPR, P, 2, REFW], F8,
                           kind="ExternalInput")
    tgtT8 = nc.dram_tensor("tgtT8", [KCP, P, 2, R_q], F8,
                           kind="ExternalInput")
    kvwk8 = nc.dram_tensor("kvwk8", [KCP, P, 2, C], F8, kind="ExternalInput")
    qw8 = nc.dram_tensor("qw8", [KCP, P, 2, C], F8, kind="ExternalInput")
    wvT8 = nc.dram_tensor("wvT8", [KCP, P, 2, C], F8, kind="ExternalInput")
    pwT_bf = nc.dram_tensor("pwT_bf", [KC, P, C], BF, kind="ExternalInput")
    E_const = nc.dram_tensor("E_const", [KC, H, P], BF, kind="ExternalInput")
    out_ext = nc.dram_tensor("out", [R_q, C], BF, kind="ExternalOutput")
    cc_in_a = nc.dram_tensor("cc_in_a", [STATE], BF)
    cc_out_a = nc.dram_tensor("cc_out_a", [STATE], BF)
    cc_in_b = nc.dram_tensor("cc_in_b", [STATE], BF)
    cc_out_b = nc.dram_tensor("cc_out_b", [STATE], BF)

    def two(t, f=None):
        # [P, 2*F] tile -> [P, 2, F] view (DoubleRow operand layout)
        return t[:].rearrange("p (two f) -> p two f", two=2)

    with TileContext(nc) as tc:
        with (
            tc.tile_pool(name="res", bufs=1) as res,
            tc.tile_pool(name="mm", bufs=3, space="PSUM") as pmm,
            tc.tile_pool(name="k8", bufs=3) as k8p,
            tc.tile_pool(name="tmp", bufs=8) as tmp,
            tc.tile_pool(name="rc", bufs=3) as rcp,
            tc.tile_pool(name="qte", bufs=1) as qtep,
            tc.tile_pool(name="xt", bufs=2 * (1 + lookahead)) as xtp,
            tc.tile_pool(name="o", bufs=8) as op_,
        ):
            # ---- resident inputs ----
            # k-weights + the first ref pieces first so phase 1 starts
            # after ~0.8MB of DMA; ref8n pair pc feeds the same row tiles
            # as refT8 piece pc.
            NPIECE = 16
            PC_R = R_ref // NPIECE
            PC_Q = R_q // NPIECE
            # Queue split: SP = kvwk + refT, ACT = ref8n, Pool-SWDGE =
            # qw + tgtT + the drain-time residents.  Each hwdge queue is
            # ~90GB/s; one queue alone starves phase 1.
            kvwk_sb = []
            for kcp in range(KCP):
                t = res.tile([P, 2 * C], F8, tag=f"kvwk{kcp}")
                nc.sync.dma_start(two(t), kvwk8[kcp])
                kvwk_sb.append(t)
            qw_sb = []
            for kcp in range(KCP):
                t = res.tile([P, 2 * C], F8, tag=f"qw{kcp}")
                nc.gpsimd.dma_start(two(t), qw8[kcp])
                qw_sb.append(t)
            refT_sb = [res.tile([P, 2 * R_ref], F8, tag=f"refT{kcp}",
                                name=f"refT_sb{kcp}") for kcp in range(KCP)]
            ref8n_sb = [res.tile([P, 2 * REFW], F8, tag=f"ref8n{pr}",
                                 name=f"ref8n_sb{pr}") for pr in range(NPR)]
            tgtT_sb = [res.tile([P, 2 * R_q], F8, tag=f"tgtT{kcp}",
                                name=f"tgtT_sb{kcp}") for kcp in range(KCP)]
            for pc in range(NPIECE):
                for kcp in range(KCP):
                    nc.sync.dma_start(two(refT_sb[kcp])[:, :, ts(pc, PC_R)],
                                      refT8[kcp][:, :, ts(pc, PC_R)])
                    nc.gpsimd.dma_start(two(tgtT_sb[kcp])[:, :, ts(pc, PC_Q)],
                                        tgtT8[kcp][:, :, ts(pc, PC_Q)])
                nc.scalar.dma_start(two(ref8n_sb[pc]), ref8n[pc])
            wvT_sb = []
            for kcp in range(KCP):
                t = res.tile([P, 2 * C], F8, tag=f"wvT{kcp}")
                nc.gpsimd.dma_start(two(t), wvT8[kcp])
                wvT_sb.append(t)
            pwT_sb = []
            for p4 in range(KC):
                t = res.tile([P, C], BF, tag=f"pwT{p4}")
                nc.gpsimd.dma_start(t[:], pwT_bf[p4])
                pwT_sb.append(t)
            E_sb = []
            for mc in range(KC):
                e = res.tile([H, P], BF, tag=f"E{mc}")
                nc.gpsimd.dma_start(e[:], E_const[mc])
                E_sb.append(e)

            # zero-init of collective-dependent tiles (no deps -> hoisted)
            Ksel = []
            for kc in range(KC):
                s = res.tile([P, H], BF, tag=f"Ksel{kc}", name=f"Ksel{kc}")
                nc.vector.memset(s[:], 0.0)
                Ksel.append(s)
            bd = []
            for p in range(NPAIR):
                t = res.tile([P, P], BF, tag=f"bd{p}", name=f"bd{p}")
                nc.vector.memset(t[:], 0.0)
                bd.append(t)
            U8 = [res.tile([P, 2 * C], F8, tag=f"U8{kcp}", name=f"U8{kcp}")
                  for kcp in range(KCP)]
            Gt8 = [res.tile([P, 2 * C], F8, tag=f"Gt8{kcp}",
                            name=f"Gt8{kcp}") for kcp in range(KCP)]

            lnws = res.tile([P, 1], F32, tag="lnws")
            nc.vector.memset(lnws[:], LN_WS)

            # ---- phase 2a: q projection + elu (collective-independent) ----
            qte2 = [[None] * KCP for _ in range(NCH)]

            def qt_chunk(j):
                for g in range(KCP):
                    q2 = qtep.tile([P, 2 * CH], BF, tag=f"qte{j}_{g}",
                                   name=f"qte{j}_{g}")
                    qte2[j][g] = q2
                    for h in range(2):
                        mc = 2 * g + h
                        pq = pmm.tile([P, CH], F32, tag="mm")
                        for kcp in range(KCP):
                            nc.tensor.matmul(pq[:],
                                             two(qw_sb[kcp])[:, :, ts(mc, P)],
                                             two(tgtT_sb[kcp])[:, :,
                                                               ts(j, CH)],
                                             start=(kcp == 0),
                                             stop=(kcp == KCP - 1),
                                             perf_mode=DR)
                        sl = slice(h * CH, (h + 1) * CH)
                        e2 = tmp.tile([P, CH], BF, tag="ex")
                        nc.scalar.activation(e2[:], pq[:],
                                             mybir.ActivationFunctionType.Exp,
                                             scale=1.0 / WS, bias=lnws[:])
                        nc.vector._custom_dve(ELU_SEL, out=q2[:, sl],
                                              in0=pq[:], in1=e2[:],
                                              s0=0.0, s1=WS)

            # ---- phase 1: k = WS*elu1(ref@Wk.T/WS), Gt/ksum accumulate ----
            pgt = tc.alloc_tile_pool(name="gt", bufs=1, space="PSUM")
            pks = tc.alloc_tile_pool(name="ks", bufs=1, space="PSUM")
            HALF1 = NPR // 2

            def drain_and_reduce(gt_ps, ks_ps, cc_in, cc_out):
                with tc.high_priority():
                    for blk in range(KC):
                        kcp, po = divmod(blk, 2)
                        nc.scalar.activation(
                            two(Gt8[kcp])[:, po, :], gt_ps[blk][:],
                            mybir.ActivationFunctionType.Copy, scale=SG)
                    ctxT_ps = pmm.tile([P, C], F32, tag="mm", name="ctxT_ps")
                    for p in range(NPAIR):
                        for kcp in range(KCP):
                            nc.tensor.matmul(
                                ctxT_ps[:, ts(p, P)],
                                two(wvT_sb[kcp])[:, :, ts(p, P)],
                                two(Gt8[kcp])[:, :, ts(p, P)],
                                start=(kcp == 0), stop=(kcp == KCP - 1),
                                perf_mode=DR)
                    ctxT_sb = rcp.tile([P, C], BF, tag="ccs",
                                       name="ctxT_sb")
                    nc.scalar.activation(ctxT_sb[:], ctxT_ps[:],
                                         mybir.ActivationFunctionType.Copy)
                    k_sb = rcp.tile([1, C], BF, tag="kss", name="kss")
                    nc.vector.tensor_copy(k_sb[:], ks_ps[:])
                    nc.sync.dma_start(
                        cc_in[0 : P * C].rearrange("(p f) -> p f", p=P),
                        ctxT_sb[:])
                    nc.sync.dma_start(
                        cc_in[P * C :].rearrange("(p f) -> p f", p=1),
                        k_sb[:])
                    nc.gpsimd.collective_compute(
                        "AllReduce", mybir.AluOpType.add,
                        replica_groups=replica_groups,
                        ins=[cc_in[:]], outs=[cc_out[:]])

            gt_ps = None
            ks_ps = None
            for i in range(NT1):
                pr, po = divmod(i, 2)
                if i % (2 * HALF1) == 0:
                    gt_ps = [pgt.tile([P, C], F32, tag=f"gt{blk}",
                                      name=f"gt_ps{blk}")
                             for blk in range(KC)]
                    ks_ps = pks.tile([1, C], F32, tag="ks", name="ks_ps")
                pk = pmm.tile([P, C], F32, tag="mm")
                for kcp in range(KCP):
                    nc.tensor.matmul(pk[:],
                                     two(refT_sb[kcp])[:, :, ts(i, P)],
                                     two(kvwk_sb[kcp]),
                                     start=(kcp == 0), stop=(kcp == KCP - 1),
                                     perf_mode=DR)
                e1 = tmp.tile([P, C], BF, tag="ex")
                nc.scalar.activation(e1[:], pk[:],
                                     mybir.ActivationFunctionType.Exp,
                                     scale=1.0 / WS, bias=lnws[:])
                if po == 0:
                    k8 = k8p.tile([P, 2 * C], F8, tag="k8", name=f"k8_{pr}")
                nc.vector._custom_dve(ELU_SEL, out=two(k8)[:, po, :],
                                      in0=pk[:], in1=e1[:], s0=0.0, s1=WS)
                if po == 1:
                    half_i = pr % HALF1
                    for blk in range(KC):
                        nc.tensor.matmul(gt_ps[blk][:],
                                         two(ref8n_sb[pr])[:, :, ts(blk, P)],
                                         two(k8),
                                         start=(half_i == 0),
                                         stop=(half_i == HALF1 - 1),
                                         perf_mode=DR)
                    nc.tensor.matmul(ks_ps[:],
                                     two(ref8n_sb[pr])[:, :, C : C + 1],
                                     two(k8),
                                     start=(half_i == 0),
                                     stop=(half_i == HALF1 - 1),
                                     perf_mode=DR)
                    if half_i == HALF1 - 1:
                        if pr < HALF1:
                            drain_and_reduce(gt_ps, ks_ps, cc_in_a, cc_out_a)
                        else:
                            drain_and_reduce(gt_ps, ks_ps, cc_in_b, cc_out_b)
            pks.release()
            pgt.release()
            for j in range(NCH):
                qt_chunk(j)

            def build_state():
                # collective results -> Ksel slivers, block-diag ctxT, U8
                with tc.high_priority():
                    ctxr = res.tile([P, C], BF, tag="ctxr", name="ctxr")
                    ctxrb = res.tile([P, C], BF, tag="ctxrb", name="ctxrb")
                    nc.sync.dma_start(
                        ctxr[:],
                        cc_out_a[0 : P * C].rearrange("(p f) -> p f", p=P))
                    nc.sync.dma_start(
                        ctxrb[:],
                        cc_out_b[0 : P * C].rearrange("(p f) -> p f", p=P))
                    ksq_a = res.tile([P, KC], BF, tag="ksqa", name="ksqa")
                    ksq_b = res.tile([P, KC], BF, tag="ksqb", name="ksqb")
                    for kc in range(KC):
                        sl = slice(P * C + kc * P, P * C + (kc + 1) * P)
                        nc.sync.dma_start(
                            ksq_a[:, kc : kc + 1],
                            cc_out_a[sl].rearrange("(p o) -> p o", o=1))
                        nc.sync.dma_start(
                            ksq_b[:, kc : kc + 1],
                            cc_out_b[sl].rearrange("(p o) -> p o", o=1))
                    ksq = res.tile([P, KC], BF, tag="ksq", name="ksq")
                    nc.vector.tensor_add(ksq[:], ksq_a[:], ksq_b[:])
                    for kc in range(KC):
                        nc.vector.tensor_copy(
                            Ksel[kc][0:D, 2 * kc : 2 * kc + 1],
                            ksq[0:D, kc : kc + 1])
                        nc.vector.tensor_copy(
                            Ksel[kc][D:P, 2 * kc + 1 : 2 * kc + 2],
                            ksq[D:P, kc : kc + 1])
                    ctxs = res.tile([P, C], BF, tag="ctxs", name="ctxs")
                    nc.vector.tensor_add(ctxs[:], ctxr[:], ctxrb[:])
                    for p in range(NPAIR):
                        nc.vector.tensor_copy(
                            bd[p][0:D, 0:D], ctxs[0:D, p * P : p * P + D])
                        nc.vector.tensor_copy(
                            bd[p][D:P, D:P],
                            ctxs[D:P, p * P + D : (p + 1) * P])
                    # U = bd @ PwT per pair -> fp8 DR tiles
                    for p in range(NPAIR):
                        u_ps = pmm.tile([P, C], F32, tag="mm", name="u_ps")
                        nc.tensor.matmul(u_ps[:], bd[p][:], pwT_sb[p][:],
                                         start=True, stop=True)
                        kcp, po = divmod(p, 2)
                        nc.scalar.activation(
                            two(U8[kcp])[:, po, :], u_ps[:],
                            mybir.ActivationFunctionType.Copy, scale=SU)

            # ---- phase 2b: den/rec/prb/mul then fused out GEMM ----
            pden = tc.alloc_tile_pool(name="den", bufs=2, space="PSUM")
            pprb = tc.alloc_tile_pool(name="prb", bufs=3, space="PSUM")

            def stage_a(j):
                den = pden.tile([H, CH], F32, tag="den", name="den")
                for kc in range(KC):
                    g, h = divmod(kc, 2)
                    nc.tensor.matmul(den[:], Ksel[kc][:],
                                     qte2[j][g][:, h * CH : (h + 1) * CH],
                                     start=(kc == 0), stop=(kc == KC - 1))
                rec = rcp.tile([H, CH], F32, tag="rec")
                nc.vector.reciprocal_approx_fast(rec[:], den[:])
                recb = rcp.tile([H, CH], BF, tag="recb")
                nc.scalar.activation(recb[:], rec[:],
                                     mybir.ActivationFunctionType.Copy)
                q8s = [xtp.tile([P, 2 * CH], F8, tag=f"xt{g}",
                                name=f"xt{g}") for g in range(KCP)]
                for mc in range(KC):
                    g, h = divmod(mc, 2)
                    prb = pprb.tile([P, CH], F32, tag="prb", name="prb")
                    nc.tensor.matmul(prb[:], E_sb[mc][:], recb[:],
                                     start=True, stop=True)
                    nc.vector.tensor_mul(
                        two(q8s[g])[:, h, :],
                        qte2[j][g][:, h * CH : (h + 1) * CH], prb[:])
                return q8s

            def stage_b(j, q8s):
                for rt in range(RT):
                    po_ = pmm.tile([P, C], F32, tag="mm", name="po")
                    for g in range(KCP):
                        nc.tensor.matmul(po_[:],
                                         two(q8s[g])[:, :, ts(rt, P)],
                                         two(U8[g]),
                                         start=(g == 0), stop=(g == 1),
                                         perf_mode=DR)
                    o_sb = op_.tile([P, C], BF, tag="o")
                    if rt == 3:
                        nc.vector.tensor_scalar_mul(o_sb[:], po_[:], DRAIN_S)
                    else:
                        nc.scalar.activation(
                            o_sb[:], po_[:],
                            mybir.ActivationFunctionType.Copy, scale=DRAIN_S)
                    nc.gpsimd.dma_start(out_ext[ts(j * RT + rt, P), :],
                                        o_sb[:])

            build_state()
            pend = []
            for j in range(NCH):
                if j < 2:
                    with tc.high_priority(offset=10**6):
                        pend.append((j, stage_a(j)))
                else:
                    pend.append((j, stage_a(j)))
                la = lookahead if j < NCH // 2 else 2
                while len(pend) > la:
                    jj, xx = pend.pop(0)
                    stage_b(jj, xx)
            for jj, xx in pend:
                stage_b(jj, xx)
            pprb.release()
            pden.release()
    nc.compile()
    return nc


def _pack_pair(mat, scale=1.0, dtype=None):
    """[C, F] -> [KCP=2, 128, 2, F]: row r = kcp*256 + po*128 + pi goes to
    [kcp, pi, po, :]."""
    Crows, F = mat.shape
    assert Crows == C
    m = (np.asarray(mat, dtype=np.float32) * scale).reshape(2, 2, 128, F)
    m = np.ascontiguousarray(m.transpose(0, 2, 1, 3))  # [kcp, pi, po, F]
    if dtype == ml_dtypes.float8_e4m3:
        m = np.clip(m, -240.0, 240.0)
    return m.astype(dtype)


def _pack_rows(mat, R, dtype):
    """[R, W] -> [R//256, 128, 2, W]: row n = pr*256 + po*128 + pi goes to
    [pr, pi, po, :]."""
    W = mat.shape[1]
    m = np.asarray(mat, dtype=np.float32).reshape(R // 256, 2, 128, W)
    m = np.ascontiguousarray(m.transpose(0, 2, 1, 3))
    m = np.clip(m, -240.0, 240.0)
    return m.astype(dtype)


def _shard_inputs(target_data, reference_data, q_w, kv_w, proj_w, proj_b,
                  R, ncores):
    bf = ml_dtypes.bfloat16
    f8 = ml_dtypes.float8_e4m3
    kv_wT = np.ascontiguousarray(np.asarray(kv_w, dtype=np.float32).T)
    kvwk8 = _pack_pair(kv_wT[:, 0:C], WS, f8)
    wvT8 = _pack_pair(kv_wT[:, C : 2 * C], WS, f8)
    qw8 = _pack_pair(np.ascontiguousarray(np.asarray(q_w).T), WS, f8)
    pwT = np.ascontiguousarray(np.asarray(proj_w, dtype=np.float32).T)
    pwT_bf = pwT.reshape(KC_, 128, C).astype(bf)
    E_const = np.zeros((KC_, H, 128), dtype=bf)
    for mc in range(KC_):
        E_const[mc, 2 * mc, 0:D] = SCALE * E0
        E_const[mc, 2 * mc + 1, D:128] = SCALE * E0
    in_maps = []
    for c in range(ncores):
        b, half = divmod(c, 2)
        sl = slice(half * R, (half + 1) * R)
        refc = np.asarray(reference_data)[b, sl, :]
        refT = refc.T  # [C, R]
        tgtT = np.asarray(target_data)[b, sl, :].T
        refn = np.zeros((R, REFW), dtype=np.float32)
        refn[:, 0:C] = refc
        refn[:, C] = 1.0
        in_maps.append({
            "refT8": _pack_pair(refT, 1.0, f8),
            "ref8n": _pack_rows(refn, R, f8),
            "tgtT8": _pack_pair(tgtT, 1.0, f8),
            "kvwk8": kvwk8, "wvT8": wvT8, "qw8": qw8,
            "pwT_bf": pwT_bf, "E_const": E_const,
        })
    return in_maps


KC_ = C // 128


def kernel(target_data, reference_data, q_w, kv_w, proj_w, proj_b):
    R = M // 2
    key = (R, NCORES)
    if key not in _CACHE:
        _CACHE[key] = build(R, R, NCORES,
                            [[0, 1], [2, 3], [4, 5], [6, 7]], lookahead=4)
    nc = _CACHE[key]
    in_maps = _shard_inputs(target_data, reference_data, q_w, kv_w, proj_w,
                            proj_b, R, NCORES)
    res = run_bass_kernel_spmd(nc, in_maps, list(range(NCORES)))
    out = np.empty((B, M, C), dtype=np.float32)
    for c in range(NCORES):
        b, half = divmod(c, 2)
        out[b, half * R : (half + 1) * R, :] = np.asarray(
            res.results[c]["out"]).astype(np.float32)
    out += np.asarray(proj_b, dtype=np.float32)[None, None, :]
    return out
